# revision 54
# baseline (speedup 1.0000x reference)
"""Trainium2 Bass kernel for a 2-layer transformer encoder.

Model (matching the reference):
    x = tok_emb[seq] + pos_emb                      # [S, E]
    for l in 0..L-1:
        x = LN(x, g[l], b[l])
        q,k,v = per-head projections of x           # [H, S, HS]
        att = softmax(q k^T / sqrt(HS))             # [H, S, S]  (output!)
        x = x + concat_heads(att @ v) @ Wr[l] + br[l]
        x = LN(x, g[l], b[l])
        x = x + relu(x @ W1[l] + b1[l]) @ W2[l] + b2[l]
    returns (x, stacked att maps)

Distribution strategy (8 cores): sequence-parallel. Core c owns query rows
[c*S/8, (c+1)*S/8). Embedding lookup is done on host as part of input
sharding. Per layer each core computes LN/QKV for its rows, the small k/v
tensors are AllGather'd across cores, then each core computes attention for
all heads restricted to its query rows, and the full FFN for its rows.
No all-reduce is needed anywhere.

On-chip layout: attention scores are computed transposed, s^T[t, s_local],
so that the AV contraction (over t) has t on the partition axis with no
transposes. The softmax denominator rides along the AV matmul via an extra
ones-column on v. Attention maps are written transposed per core and
reassembled (concat over s + transpose) on host.

All matmuls run in bf16 with f32 PSUM accumulation. LayerNorm, softmax
normalization and residuals are f32.
"""

import numpy as np
import ml_dtypes
from contextlib import ExitStack
from dataclasses import dataclass

import concourse.bass as bass
import concourse.mybir as mybir
import concourse.tile as tile
from concourse import bacc

F32 = mybir.dt.float32
BF16 = mybir.dt.bfloat16
AF = mybir.ActivationFunctionType
EPS = 1e-5


@dataclass(frozen=True)
class Cfg:
    NC: int = 8        # cores
    S: int = 2048      # full sequence
    E: int = 1024      # embed dim
    H: int = 16        # heads
    HS: int = 64       # head size
    FF: int = 4096     # ffn hidden
    L: int = 2         # layers

    @property
    def SL(self):  # local sequence rows per core
        return self.S // self.NC

    @property
    def ST(self):  # local s-tiles of 128
        return self.SL // 128

    @property
    def TT(self):  # global t-tiles of 128
        return self.S // 128

    @property
    def EK(self):  # contraction tiles over E
        return self.E // 128

    @property
    def FK(self):  # hidden tiles over FF
        return self.FF // 128

    @property
    def NP(self):  # head pairs
        return self.H // 2


CFG_FULL = Cfg()


def build_encoder(nc, cfg: Cfg, fake_cc=False):
    # fake_cc: replace AllGathers with equivalent local DMA copies so the
    # program is single-core simulatable (TimelineSim); timing-model only.
    c = cfg
    SL, ST, TT, EK, FK, NP = c.SL, c.ST, c.TT, c.EK, c.FK, c.NP

    def allgather(in_t, out_t):
        if fake_cc:
            for blk in range(c.NC):
                nc.sync.dma_start(out=out_t[blk], in_=in_t[:])
        else:
            nc.gpsimd.collective_compute(
                "AllGather",
                mybir.AluOpType.bypass,
                replica_groups=[list(range(c.NC))],
                ins=[in_t.ap().opt()],
                outs=[out_t.ap().opt()],
            )

    # ---- I/O -------------------------------------------------------------
    x0_d = nc.dram_tensor("x0", [SL, c.E], F32, kind="ExternalInput")
    wq_d = nc.dram_tensor("wq", [c.L, NP, c.E, 128], BF16, kind="ExternalInput")
    wk_d = nc.dram_tensor("wk", [c.L, NP, c.E, 128], BF16, kind="ExternalInput")
    wv_d = nc.dram_tensor("wv", [c.L, NP, c.E, 128], BF16, kind="ExternalInput")
    wr_d = nc.dram_tensor("wr", [c.L, c.E, c.E], BF16, kind="ExternalInput")
    w1_d = nc.dram_tensor("w1", [c.L, c.E, c.FF], BF16, kind="ExternalInput")
    w2_d = nc.dram_tensor("w2", [c.L, c.FF, c.E], BF16, kind="ExternalInput")
    lng_d = nc.dram_tensor("lng", [c.L, c.E], F32, kind="ExternalInput")
    lnb_d = nc.dram_tensor("lnb", [c.L, c.E], F32, kind="ExternalInput")
    br_d = nc.dram_tensor("br", [c.L, c.E], F32, kind="ExternalInput")
    b1_d = nc.dram_tensor("b1", [c.L, c.FF], F32, kind="ExternalInput")
    b2_d = nc.dram_tensor("b2", [c.L, c.E], F32, kind="ExternalInput")

    xout_d = nc.dram_tensor("x_out", [SL, c.E], F32, kind="ExternalOutput")
    # transposed att maps: att_t[l, h, t, s_local]
    att_d = nc.dram_tensor("att_t", [c.L, c.H, c.S, SL], BF16, kind="ExternalOutput")

    # collective bounce buffers (internal DRAM), one pair per layer
    cc_space = "Shared" if (c.NC > 4 and not fake_cc) else "Local"
    k_in = [
        nc.dram_tensor(f"k_in{l}", [NP, 128, SL], BF16) for l in range(c.L)
    ]
    k_out = [
        nc.dram_tensor(f"k_out{l}", [c.NC, NP, 128, SL], BF16, addr_space=cc_space)
        for l in range(c.L)
    ]
    v_in = [
        nc.dram_tensor(f"v_in{l}", [NP, 2, ST, 128, 65], BF16) for l in range(c.L)
    ]
    v_out = [
        nc.dram_tensor(
            f"v_out{l}", [c.NC, NP, 2, ST, 128, 65], BF16, addr_space=cc_space
        )
        for l in range(c.L)
    ]

    with tile.TileContext(nc) as tc, ExitStack() as ctx:
        pool = lambda name, bufs, space="SBUF": ctx.enter_context(
            tc.tile_pool(name=name, bufs=bufs, space=space)
        )

        consts = pool("consts", 1)
        xA = pool("xA", 2 * ST + 2)          # x / x2 / x3 residual stream f32
        xB = pool("xB", 2 * ST + 2)          # LN outputs f32
        xbfp = pool("xbf", ST + 2)           # LN outputs cast to bf16
        xTp = pool("xT", 2 * EK + 2)         # transposed LN outputs bf16
        bvec = pool("bvec", 1)               # broadcast bias/gain vectors f32
        wqkv = pool("wqkv", 2)               # qkv weight tiles
        wmisc = pool("wmisc", 5)             # w1/w2/wr tiles
        kvloc = pool("kvloc", 8)             # local k/v_aug tiles pre-AG
        qp = pool("q", NP + 1)               # q^T pair tiles bf16
        kp = pool("k", 2)                    # gathered k^T pair tiles bf16
        vp = pool("v", 6)                    # gathered v_aug head tiles bf16
        ep = pool("e", 5)                    # per-head exp(scores) bf16
        rsp = pool("rs", 3)                  # rowsum scratch
        rbp = pool("rb", 3)                  # broadcast recip tiles
        catp = pool("cat", c.H + 1)          # normalized av (cat) tiles bf16
        hp = pool("h", 4)                    # ffn hidden tiles bf16

        # single PSUM pool: 8 slots of one bank each, all tiles share one
        # tag so slots are grabbed from the free pool as tiles come alive
        psp = pool("psp", 8, space="PSUM")

        eps_t = consts.tile([128, 1], F32)
        nc.vector.memset(eps_t, EPS)

        NCH = (c.E + 511) // 512  # layernorm bn_stats chunks
        CHM = min(c.E, 512)

        def layernorm(x_tiles, g_bc, b_bc):
            """returns new f32 tiles from xB: LN(x) * g + b"""
            out = []
            for st in range(ST):
                xt = x_tiles[st]
                stats = rsp.tile([128, NCH, 6], F32, tag="ln_stats")
                xr = xt.rearrange("p (c f) -> p c f", f=CHM)
                for ch in range(NCH):
                    nc.vector.bn_stats(out=stats[:, ch, :], in_=xr[:, ch, :])
                mv = rsp.tile([128, 2], F32, tag="ln_mv")
                nc.vector.bn_aggr(out=mv, in_=stats)
                rstd = rsp.tile([128, 1], F32, tag="ln_rstd")
                nc.scalar.activation(
                    out=rstd, in_=mv[:, 1:2], func=AF.Sqrt, bias=eps_t, scale=1.0
                )
                nc.vector.reciprocal(out=rstd, in_=rstd)
                xn = xB.tile([128, c.E], F32, tag="xn")
                # (x - mu) * g  then  (prev * rstd) + b   (2 fused DVE ops)
                nc.vector.scalar_tensor_tensor(
                    out=xn,
                    in0=xt,
                    scalar=mv[:, 0:1],
                    in1=g_bc,
                    op0=mybir.AluOpType.subtract,
                    op1=mybir.AluOpType.mult,
                )
                nc.vector.scalar_tensor_tensor(
                    out=xn,
                    in0=xn,
                    scalar=rstd,
                    in1=b_bc,
                    op0=mybir.AluOpType.mult,
                    op1=mybir.AluOpType.add,
                )
                out.append(xn)
            return out

        def transpose_to(xn_tiles):
            """xn f32 [ST][128, E] -> bf16 cast -> xT [EK][128, SL] bf16"""
            xbf = []
            for st in range(ST):
                t = xbfp.tile([128, c.E], BF16, tag="xbf")
                nc.vector.tensor_copy(out=t, in_=xn_tiles[st])
                xbf.append(t)
            xT = []
            for kk in range(EK):
                xt = xTp.tile([128, SL], BF16, tag="xT")
                for st in range(ST):
                    nc.sync.dma_start(
                        out=xt[:, st * 128 : (st + 1) * 128],
                        in_=xbf[st][:, kk * 128 : (kk + 1) * 128],
                        transpose=True,
                    )
                xT.append(xt)
            return xT

        # ---- load x0 ----------------------------------------------------
        x_tiles = []
        for st in range(ST):
            xt = xA.tile([128, c.E], F32, tag="x")
            nc.sync.dma_start(out=xt, in_=x0_d[st * 128 : (st + 1) * 128, :])
            x_tiles.append(xt)

        for l in range(c.L):
            # broadcast per-layer vectors
            g_bc = bvec.tile([128, c.E], F32, tag="g_bc")
            b_bc = bvec.tile([128, c.E], F32, tag="b_bc")
            br_bc = bvec.tile([128, c.E], F32, tag="br_bc")
            b2_bc = bvec.tile([128, c.E], F32, tag="b2_bc")
            for dst, srcd in ((g_bc, lng_d), (b_bc, lnb_d), (br_bc, br_d),
                              (b2_bc, b2_d)):
                nc.sync.dma_start(out=dst[0:1, :], in_=srcd.ap()[l])
                nc.gpsimd.partition_broadcast(dst[:, :], dst[0:1, :])
            b1_sb = bvec.tile([128, FK], F32, tag="b1_sb")
            nc.sync.dma_start(
                out=b1_sb, in_=b1_d.ap()[l].rearrange("(t p) -> p t", p=128)
            )

            # ---- LN1 + transposed activations ---------------------------
            xn1 = layernorm(x_tiles, g_bc, b_bc)
            xT1 = transpose_to(xn1)

            # ---- k projections first, then AG(k) ------------------------
            for p in range(NP):
                wk_sb = wqkv.tile([128, EK, 128], BF16, tag="wk_sb")
                nc.sync.dma_start(
                    out=wk_sb,
                    in_=wk_d.ap()[l, p].rearrange("(k p) m -> p k m", p=128),
                )
                kps = psp.tile([128, SL], F32, tag="ps", name="qkps")
                for kk in range(EK):
                    nc.tensor.matmul(
                        kps, wk_sb[:, kk, :], xT1[kk],
                        start=(kk == 0), stop=(kk == EK - 1),
                    )
                kloc = kvloc.tile([128, SL], BF16, tag="kloc")
                nc.vector.tensor_copy(out=kloc, in_=kps)
                nc.sync.dma_start(out=k_in[l][p], in_=kloc)
            allgather(k_in[l], k_out[l])

            # ---- q and v projections, then AG(v) ------------------------
            q_sb = []
            for p in range(NP):
                wq_sb = wqkv.tile([128, EK, 128], BF16, tag="wq_sb")
                nc.sync.dma_start(
                    out=wq_sb,
                    in_=wq_d.ap()[l, p].rearrange("(k p) m -> p k m", p=128),
                )
                qps = psp.tile([128, SL], F32, tag="ps", name="qkps")
                for kk in range(EK):
                    nc.tensor.matmul(
                        qps, wq_sb[:, kk, :], xT1[kk],
                        start=(kk == 0), stop=(kk == EK - 1),
                    )
                qt = qp.tile([128, SL], BF16, tag="q")
                nc.vector.tensor_copy(out=qt, in_=qps)
                q_sb.append(qt)

                wv_sb = wqkv.tile([128, EK, 128], BF16, tag="wv_sb")
                nc.sync.dma_start(
                    out=wv_sb,
                    in_=wv_d.ap()[l, p].rearrange("(k p) m -> p k m", p=128),
                )
                for st in range(ST):
                    vps = psp.tile([128, 128], F32, tag="ps", name="vps")
                    for kk in range(EK):
                        nc.tensor.matmul(
                            vps,
                            xT1[kk][:, st * 128 : (st + 1) * 128],
                            wv_sb[:, kk, :],
                            start=(kk == 0), stop=(kk == EK - 1),
                        )
                    for hh in range(2):
                        vaug = kvloc.tile([128, 65], BF16, tag="vaug")
                        nc.vector.tensor_copy(
                            out=vaug[:, 0:64], in_=vps[:, hh * 64 : hh * 64 + 64]
                        )
                        nc.vector.memset(vaug[:, 64:65], 1.0)
                        nc.sync.dma_start(out=v_in[l][p, hh, st], in_=vaug)
            allgather(v_in[l], v_out[l])

            # ---- attention ----------------------------------------------
            cat_tiles = []
            for p in range(NP):
                # gathered k^T for this pair: [128, S] over core blocks
                k_sb = kp.tile([128, c.NC, SL], BF16, tag="k_sb")
                nc.sync.dma_start(
                    out=k_sb, in_=k_out[l][:, p].transpose([1, 0, 2])
                )
                k_flat = k_sb.rearrange("p c s -> p (c s)")
                for hh in range(2):
                    h_idx = 2 * p + hh
                    hs = slice(hh * 64, hh * 64 + 64)
                    v_sb = vp.tile([128, c.NC, ST, 65], BF16, tag="v_sb")
                    for t2 in range(ST):
                        nc.sync.dma_start(
                            out=v_sb[:, :, t2, :],
                            in_=v_out[l][:, p, hh, t2].transpose([1, 0, 2]),
                        )
                    v_flat = v_sb.rearrange("p c t f -> p (c t) f")
                    avps = psp.tile([65, SL], F32, tag="ps", name="avps")
                    e_sb = ep.tile([128, TT, SL], BF16, tag="e")
                    for tp2 in range(TT // 2):
                        scps = psp.tile([128, 2 * SL], F32, tag="ps", name="scps")
                        for half in range(2):
                            tt = 2 * tp2 + half
                            nc.tensor.matmul(
                                scps[:, half * SL : (half + 1) * SL],
                                k_flat[hs, tt * 128 : (tt + 1) * 128],
                                q_sb[p][hs, :],
                            )
                        nc.scalar.activation(
                            out=e_sb[:, 2 * tp2 : 2 * tp2 + 2, :], in_=scps,
                            func=AF.Exp, scale=1.0 / np.sqrt(c.HS),
                        )
                        for half in range(2):
                            tt = 2 * tp2 + half
                            nc.tensor.matmul(
                                avps, v_flat[:, tt, :], e_sb[:, tt, :],
                                start=(tt == 0), stop=(tt == TT - 1),
                            )
                    # rowsums -> reciprocal -> partition-broadcast (SBUF DMA)
                    rs = rsp.tile([128, SL], F32, tag="rs")
                    nc.vector.reciprocal(out=rs[64:65, :], in_=avps[64:65, :])
                    rsb = rsp.tile([128, SL], BF16, tag="rsb")
                    nc.vector.tensor_copy(out=rsb[64:65, :], in_=rs[64:65, :])
                    rb = rbp.tile([128, SL], BF16, tag="rb")
                    nc.sync.dma_start(out=rb[0:1, :], in_=rsb[64:65, :])
                    nc.gpsimd.partition_broadcast(rb[:, :], rb[0:1, :])
                    # normalized av -> cat tile
                    cat = catp.tile([64, SL], BF16, tag="cat")
                    nc.vector.tensor_mul(out=cat, in0=avps[0:64, :], in1=rb[0:64, :])
                    cat_tiles.append(cat)
                    # normalized att map: in-place bcast multiply + one DMA
                    # (av matmuls finished reading e_sb before rb exists)
                    rb_rep = bass.AP(
                        tensor=rb.tensor,
                        offset=rb.offset,
                        ap=[list(rb.ap[0]), [0, TT]]
                        + [list(d) for d in rb.ap[1:]],
                    )
                    nc.vector.tensor_mul(out=e_sb, in0=e_sb, in1=rb_rep)
                    nc.sync.dma_start(
                        out=att_d[l, h_idx].rearrange("(t p) s -> p t s", p=128),
                        in_=e_sb,
                    )

            # ---- resize projection + residual ---------------------------
            for st in range(ST):
                nc.vector.tensor_add(out=xn1[st], in0=xn1[st], in1=br_bc)
            NN = c.E // 512 if c.E >= 512 else 1
            NW = min(c.E, 512)
            xatt_ps = [
                [psp.tile([128, NW], F32, tag="ps", name="xatt_ps")
                 for _ in range(NN)]
                for _ in range(ST)
            ]
            for h in range(c.H):
                for nn2 in range(NN):
                    wr_sb = wmisc.tile([64, NW], BF16, tag="wr_sb")
                    nc.sync.dma_start(
                        out=wr_sb,
                        in_=wr_d[l, h * 64 : h * 64 + 64, nn2 * NW : (nn2 + 1) * NW],
                    )
                    for st in range(ST):
                        nc.tensor.matmul(
                            xatt_ps[st][nn2],
                            cat_tiles[h][:, st * 128 : (st + 1) * 128],
                            wr_sb,
                            start=(h == 0), stop=(h == c.H - 1),
                        )
            x2_tiles = []
            for st in range(ST):
                x2 = xA.tile([128, c.E], F32, tag="x")
                for nn2 in range(NN):
                    nc.vector.tensor_add(
                        out=x2[:, nn2 * NW : (nn2 + 1) * NW],
                        in0=xatt_ps[st][nn2],
                        in1=xn1[st][:, nn2 * NW : (nn2 + 1) * NW],
                    )
                x2_tiles.append(x2)

            # ---- LN2 + FFN ----------------------------------------------
            xn2 = layernorm(x2_tiles, g_bc, b_bc)
            xT2 = transpose_to(xn2)
            for st in range(ST):
                nc.vector.tensor_add(out=xn2[st], in0=xn2[st], in1=b2_bc)

            y_ps = [
                [psp.tile([128, NW], F32, tag="ps", name="y_ps")
                 for _ in range(NN)]
                for _ in range(ST)
            ]
            for mk in range(FK):
                w1_sb = wmisc.tile([128, EK, 128], BF16, tag="w1_sb")
                nc.sync.dma_start(
                    out=w1_sb,
                    in_=w1_d[l, :, mk * 128 : (mk + 1) * 128].rearrange(
                        "(k p) m -> p k m", p=128
                    ),
                )
                hps = psp.tile([128, SL], F32, tag="ps", name="hps")
                for kk in range(EK):
                    nc.tensor.matmul(
                        hps, w1_sb[:, kk, :], xT2[kk],
                        start=(kk == 0), stop=(kk == EK - 1),
                    )
                h_sb = hp.tile([128, SL], BF16, tag="h_sb")
                nc.scalar.activation(
                    out=h_sb, in_=hps, func=AF.Relu,
                    bias=b1_sb[:, mk : mk + 1], scale=1.0,
                )
                w2_sb = wmisc.tile([128, c.E], BF16, tag="w2_sb")
                nc.sync.dma_start(
                    out=w2_sb, in_=w2_d[l, mk * 128 : (mk + 1) * 128, :]
                )
                for st in range(ST):
                    for nn2 in range(NN):
                        nc.tensor.matmul(
                            y_ps[st][nn2],
                            h_sb[:, st * 128 : (st + 1) * 128],
                            w2_sb[:, nn2 * NW : (nn2 + 1) * NW],
                            start=(mk == 0), stop=(mk == FK - 1),
                        )
            new_x = []
            for st in range(ST):
                x3 = xA.tile([128, c.E], F32, tag="x")
                for nn2 in range(NN):
                    nc.vector.tensor_add(
                        out=x3[:, nn2 * NW : (nn2 + 1) * NW],
                        in0=y_ps[st][nn2],
                        in1=xn2[st][:, nn2 * NW : (nn2 + 1) * NW],
                    )
                new_x.append(x3)
            x_tiles = new_x

        for st in range(ST):
            nc.sync.dma_start(
                out=xout_d[st * 128 : (st + 1) * 128, :], in_=x_tiles[st]
            )

    return nc


# --------------------------------------------------------------------------
# host side
# --------------------------------------------------------------------------

def make_in_maps(cfg: Cfg, seq, tok_emb, pos_emb, Wq, Wk, Wv, Wr, br, W1, b1,
                 W2, b2, ln_g, ln_b):
    c = cfg
    bf = ml_dtypes.bfloat16
    seq = np.asarray(seq)
    x0 = np.asarray(tok_emb)[seq.astype(np.int64)] + np.asarray(pos_emb)
    x0 = np.ascontiguousarray(x0, dtype=np.float32)

    # head-pair packed projection weights [L, NP, E, 2*HS]:
    # pair p columns = [head 2p | head 2p+1]
    def pairs(W):
        W = np.asarray(W)  # [L, H, E, HS]
        W = W.reshape(c.L, c.NP, 2, c.E, c.HS)
        W = np.concatenate([W[:, :, 0], W[:, :, 1]], axis=-1)  # [L, NP, E, 128]
        return np.ascontiguousarray(W, dtype=bf)

    common = {
        "wq": pairs(Wq),
        "wk": pairs(Wk),
        "wv": pairs(Wv),
        "wr": np.ascontiguousarray(np.asarray(Wr), dtype=bf),
        "w1": np.ascontiguousarray(np.asarray(W1), dtype=bf),
        "w2": np.ascontiguousarray(np.asarray(W2), dtype=bf),
        "lng": np.ascontiguousarray(np.asarray(ln_g), dtype=np.float32),
        "lnb": np.ascontiguousarray(np.asarray(ln_b), dtype=np.float32),
        "br": np.ascontiguousarray(np.asarray(br), dtype=np.float32),
        "b1": np.ascontiguousarray(np.asarray(b1), dtype=np.float32),
        "b2": np.ascontiguousarray(np.asarray(b2), dtype=np.float32),
    }
    in_maps = []
    for core in range(c.NC):
        m = dict(common)
        m["x0"] = x0[core * c.SL : (core + 1) * c.SL]
        in_maps.append(m)
    return in_maps


def assemble_outputs(cfg: Cfg, results):
    c = cfg
    x = np.concatenate(
        [np.asarray(r["x_out"], np.float32).reshape(c.SL, c.E) for r in results],
        axis=0,
    )
    att = np.empty((c.L, c.H, c.S, c.S), np.float32)
    for core, r in enumerate(results):
        blk = np.asarray(r["att_t"], np.float32).reshape(c.L, c.H, c.S, c.SL)
        att[:, :, core * c.SL : (core + 1) * c.SL, :] = blk.transpose(0, 1, 3, 2)
    return x, att


_CACHE = {}


def _get_nc(cfg: Cfg):
    key = cfg
    if key not in _CACHE:
        nc = bacc.Bacc(
            "TRN2",
            target_bir_lowering=False,
            debug=False,
            num_devices=cfg.NC,
        )
        build_encoder(nc, cfg)
        nc.compile()
        _CACHE[key] = nc
    return _CACHE[key]


LAST_RESULTS = {}


def kernel(seq, tok_emb, pos_emb, Wq, Wk, Wv, Wr, br, W1, b1, W2, b2, ln_g,
           ln_b):
    import os
    from concourse import bass_utils

    cfg = CFG_FULL
    nc = _get_nc(cfg)
    in_maps = make_in_maps(
        cfg, seq, tok_emb, pos_emb, Wq, Wk, Wv, Wr, br, W1, b1, W2, b2,
        ln_g, ln_b,
    )
    trace = bool(int(os.environ.get("KERNEL_TRACE", "0")))
    res = bass_utils.run_bass_kernel_spmd(
        nc, in_maps, core_ids=list(range(cfg.NC)), trace=trace
    )
    LAST_RESULTS["exec_time_ns"] = res.exec_time_ns
    LAST_RESULTS["profile_json"] = res.profile_json
    return assemble_outputs(cfg, res.results)


# revision 55
# speedup vs baseline: 12.9667x; 12.9667x over previous
"""Trainium2 Bass kernel for a 2-layer transformer encoder.

Model (matching the reference):
    x = tok_emb[seq] + pos_emb                      # [S, E]
    for l in 0..L-1:
        x = LN(x, g[l], b[l])
        q,k,v = per-head projections of x           # [H, S, HS]
        att = softmax(q k^T / sqrt(HS))             # [H, S, S]  (output!)
        x = x + concat_heads(att @ v) @ Wr[l] + br[l]
        x = LN(x, g[l], b[l])
        x = x + relu(x @ W1[l] + b1[l]) @ W2[l] + b2[l]
    returns (x, stacked att maps)

Distribution strategy (8 cores): sequence-parallel. Core c owns query rows
[c*S/8, (c+1)*S/8). Embedding lookup is done on host as part of input
sharding. Per layer each core computes LN/QKV for its rows, the small k/v
tensors are AllGather'd across cores, then each core computes attention for
all heads restricted to its query rows, and the full FFN for its rows.
No all-reduce is needed anywhere.

On-chip layout: attention scores are computed transposed, s^T[t, s_local],
so that the AV contraction (over t) has t on the partition axis with no
transposes. The softmax denominator rides along the AV matmul via an extra
ones-column on v. Attention maps are written transposed per core and
reassembled (concat over s + transpose) on host.

All matmuls run in bf16 with f32 PSUM accumulation. LayerNorm, softmax
normalization and residuals are f32.
"""

import numpy as np
import ml_dtypes
from contextlib import ExitStack
from dataclasses import dataclass

import concourse.bass as bass
import concourse.mybir as mybir
import concourse.tile as tile
from concourse import bacc

F32 = mybir.dt.float32
BF16 = mybir.dt.bfloat16
AF = mybir.ActivationFunctionType
EPS = 1e-5


@dataclass(frozen=True)
class Cfg:
    NC: int = 8        # cores
    S: int = 2048      # full sequence
    E: int = 1024      # embed dim
    H: int = 16        # heads
    HS: int = 64       # head size
    FF: int = 4096     # ffn hidden
    L: int = 2         # layers

    @property
    def SL(self):  # local sequence rows per core
        return self.S // self.NC

    @property
    def ST(self):  # local s-tiles of 128
        return self.SL // 128

    @property
    def TT(self):  # global t-tiles of 128
        return self.S // 128

    @property
    def EK(self):  # contraction tiles over E
        return self.E // 128

    @property
    def FK(self):  # hidden tiles over FF
        return self.FF // 128

    @property
    def NP(self):  # head pairs
        return self.H // 2


CFG_FULL = Cfg()


def build_encoder(nc, cfg: Cfg, fake_cc=False):
    # fake_cc: replace AllGathers with equivalent local DMA copies so the
    # program is single-core simulatable (TimelineSim); timing-model only.
    c = cfg
    SL, ST, TT, EK, FK, NP = c.SL, c.ST, c.TT, c.EK, c.FK, c.NP

    def allgather(in_t, out_t):
        if fake_cc:
            for blk in range(c.NC):
                nc.sync.dma_start(out=out_t[blk], in_=in_t[:])
        else:
            nc.gpsimd.collective_compute(
                "AllGather",
                mybir.AluOpType.bypass,
                replica_groups=[list(range(c.NC))],
                ins=[in_t.ap().opt()],
                outs=[out_t.ap().opt()],
            )

    # ---- I/O -------------------------------------------------------------
    x0_d = nc.dram_tensor("x0", [SL, c.E], F32, kind="ExternalInput")
    wq_d = nc.dram_tensor("wq", [c.L, NP, c.E, 128], BF16, kind="ExternalInput")
    wk_d = nc.dram_tensor("wk", [c.L, NP, c.E, 128], BF16, kind="ExternalInput")
    wv_d = nc.dram_tensor("wv", [c.L, NP, c.E, 128], BF16, kind="ExternalInput")
    wr_d = nc.dram_tensor("wr", [c.L, c.E, c.E], BF16, kind="ExternalInput")
    w1_d = nc.dram_tensor("w1", [c.L, c.E, c.FF], BF16, kind="ExternalInput")
    w2_d = nc.dram_tensor("w2", [c.L, c.FF, c.E], BF16, kind="ExternalInput")
    lng_d = nc.dram_tensor("lng", [c.L, c.E], F32, kind="ExternalInput")
    lnb_d = nc.dram_tensor("lnb", [c.L, c.E], F32, kind="ExternalInput")
    br_d = nc.dram_tensor("br", [c.L, c.E], F32, kind="ExternalInput")
    b1_d = nc.dram_tensor("b1", [c.L, c.FF], F32, kind="ExternalInput")
    b2_d = nc.dram_tensor("b2", [c.L, c.E], F32, kind="ExternalInput")

    xout_d = nc.dram_tensor("x_out", [SL, c.E], F32, kind="ExternalOutput")
    # transposed att maps: att_t[l, h, t, s_local]
    att_d = nc.dram_tensor("att_t", [c.L, c.H, c.S, SL], BF16, kind="ExternalOutput")

    # collective bounce buffers (internal DRAM), one pair per layer
    cc_space = "Shared" if (c.NC > 4 and not fake_cc) else "Local"
    k_in = [
        nc.dram_tensor(f"k_in{l}", [NP, 128, SL], BF16) for l in range(c.L)
    ]
    k_out = [
        nc.dram_tensor(f"k_out{l}", [c.NC, NP, 128, SL], BF16, addr_space=cc_space)
        for l in range(c.L)
    ]
    v_in = [
        nc.dram_tensor(f"v_in{l}", [NP, 2, ST, 128, 65], BF16) for l in range(c.L)
    ]
    v_out = [
        nc.dram_tensor(
            f"v_out{l}", [c.NC, NP, 2, ST, 128, 65], BF16, addr_space=cc_space
        )
        for l in range(c.L)
    ]

    with tile.TileContext(nc) as tc, ExitStack() as ctx:
        pool = lambda name, bufs, space="SBUF": ctx.enter_context(
            tc.tile_pool(name=name, bufs=bufs, space=space)
        )

        consts = pool("consts", 1)
        xA = pool("xA", 2 * ST + 2)          # x / x2 / x3 residual stream f32
        xB = pool("xB", 2 * ST + 2)          # LN outputs f32
        xbfp = pool("xbf", ST + 2)           # LN outputs cast to bf16
        xTp = pool("xT", 2 * EK + 2)         # transposed LN outputs bf16
        bvec = pool("bvec", 1)               # broadcast bias/gain vectors f32
        wqkv = pool("wqkv", 2)               # qkv weight tiles
        wmisc = pool("wmisc", 5)             # w1/w2/wr tiles
        kvloc = pool("kvloc", 8)             # local k/v_aug tiles pre-AG
        qp = pool("q", NP + 1)               # q^T pair tiles bf16
        kp = pool("k", 2)                    # gathered k^T pair tiles bf16
        vp = pool("v", 6)                    # gathered v_aug head tiles bf16
        ep = pool("e", 5)                    # per-head exp(scores) bf16
        rsp = pool("rs", 3)                  # rowsum scratch
        rbp = pool("rb", 4)                  # broadcast recip tiles
        catp = pool("cat", c.H + 1)          # normalized av (cat) tiles bf16
        hp = pool("h", 6)                    # ffn hidden tiles bf16

        # single PSUM pool: 8 slots of one bank each, all tiles share one
        # tag so slots are grabbed from the free pool as tiles come alive
        psp = pool("psp", 8, space="PSUM")

        eps_t = consts.tile([128, 1], F32)
        nc.vector.memset(eps_t, EPS)

        NCH = (c.E + 511) // 512  # layernorm bn_stats chunks
        CHM = min(c.E, 512)

        def layernorm(x_tiles, g_bc, b_bc):
            """returns new f32 tiles from xB: LN(x) * g + b"""
            out = []
            for st in range(ST):
                xt = x_tiles[st]
                stats = rsp.tile([128, NCH, 6], F32, tag="ln_stats")
                xr = xt.rearrange("p (c f) -> p c f", f=CHM)
                for ch in range(NCH):
                    nc.vector.bn_stats(out=stats[:, ch, :], in_=xr[:, ch, :])
                mv = rsp.tile([128, 2], F32, tag="ln_mv")
                nc.vector.bn_aggr(out=mv, in_=stats)
                rstd = rsp.tile([128, 1], F32, tag="ln_rstd")
                nc.scalar.activation(
                    out=rstd, in_=mv[:, 1:2], func=AF.Sqrt, bias=eps_t, scale=1.0
                )
                nc.vector.reciprocal(out=rstd, in_=rstd)
                xn = xB.tile([128, c.E], F32, tag="xn")
                # (x - mu) * g  then  (prev * rstd) + b   (2 fused DVE ops)
                nc.vector.scalar_tensor_tensor(
                    out=xn,
                    in0=xt,
                    scalar=mv[:, 0:1],
                    in1=g_bc,
                    op0=mybir.AluOpType.subtract,
                    op1=mybir.AluOpType.mult,
                )
                nc.vector.scalar_tensor_tensor(
                    out=xn,
                    in0=xn,
                    scalar=rstd,
                    in1=b_bc,
                    op0=mybir.AluOpType.mult,
                    op1=mybir.AluOpType.add,
                )
                out.append(xn)
            return out

        def transpose_to(xn_tiles):
            """xn f32 [ST][128, E] -> bf16 cast -> xT [EK][128, SL] bf16"""
            xbf = []
            for st in range(ST):
                t = xbfp.tile([128, c.E], BF16, tag="xbf")
                nc.vector.tensor_copy(out=t, in_=xn_tiles[st])
                xbf.append(t)
            xT = []
            for kk in range(EK):
                xt = xTp.tile([128, SL], BF16, tag="xT")
                for st in range(ST):
                    nc.sync.dma_start(
                        out=xt[:, st * 128 : (st + 1) * 128],
                        in_=xbf[st][:, kk * 128 : (kk + 1) * 128],
                        transpose=True,
                    )
                xT.append(xt)
            return xT

        # ---- load x0 ----------------------------------------------------
        x_tiles = []
        for st in range(ST):
            xt = xA.tile([128, c.E], F32, tag="x")
            nc.sync.dma_start(out=xt, in_=x0_d[st * 128 : (st + 1) * 128, :])
            x_tiles.append(xt)

        for l in range(c.L):
            # broadcast per-layer vectors
            g_bc = bvec.tile([128, c.E], F32, tag="g_bc")
            b_bc = bvec.tile([128, c.E], F32, tag="b_bc")
            br_bc = bvec.tile([128, c.E], F32, tag="br_bc")
            b2_bc = bvec.tile([128, c.E], F32, tag="b2_bc")
            for dst, srcd in ((g_bc, lng_d), (b_bc, lnb_d), (br_bc, br_d),
                              (b2_bc, b2_d)):
                nc.sync.dma_start(out=dst[0:1, :], in_=srcd.ap()[l])
                nc.gpsimd.partition_broadcast(dst[:, :], dst[0:1, :])
            b1_sb = bvec.tile([128, FK], F32, tag="b1_sb")
            nc.sync.dma_start(
                out=b1_sb, in_=b1_d.ap()[l].rearrange("(t p) -> p t", p=128)
            )

            # ---- LN1 + transposed activations ---------------------------
            xn1 = layernorm(x_tiles, g_bc, b_bc)
            xT1 = transpose_to(xn1)

            # ---- k projections first, then AG(k) ------------------------
            for p in range(NP):
                wk_sb = wqkv.tile([128, EK, 128], BF16, tag="wk_sb")
                nc.sync.dma_start(
                    out=wk_sb,
                    in_=wk_d.ap()[l, p].rearrange("(k p) m -> p k m", p=128),
                )
                kps = psp.tile([128, SL], F32, tag="ps", name="qkps")
                for kk in range(EK):
                    nc.tensor.matmul(
                        kps, wk_sb[:, kk, :], xT1[kk],
                        start=(kk == 0), stop=(kk == EK - 1),
                    )
                kloc = kvloc.tile([128, SL], BF16, tag="kloc")
                nc.vector.tensor_copy(out=kloc, in_=kps)
                nc.sync.dma_start(out=k_in[l][p], in_=kloc)
            allgather(k_in[l], k_out[l])

            # ---- q and v projections, then AG(v) ------------------------
            q_sb = []
            for p in range(NP):
                wq_sb = wqkv.tile([128, EK, 128], BF16, tag="wq_sb")
                nc.sync.dma_start(
                    out=wq_sb,
                    in_=wq_d.ap()[l, p].rearrange("(k p) m -> p k m", p=128),
                )
                qps = psp.tile([128, SL], F32, tag="ps", name="qkps")
                for kk in range(EK):
                    nc.tensor.matmul(
                        qps, wq_sb[:, kk, :], xT1[kk],
                        start=(kk == 0), stop=(kk == EK - 1),
                    )
                qt = qp.tile([128, SL], BF16, tag="q")
                nc.vector.tensor_copy(out=qt, in_=qps)
                q_sb.append(qt)

                wv_sb = wqkv.tile([128, EK, 128], BF16, tag="wv_sb")
                nc.sync.dma_start(
                    out=wv_sb,
                    in_=wv_d.ap()[l, p].rearrange("(k p) m -> p k m", p=128),
                )
                for st in range(ST):
                    vps = psp.tile([128, 128], F32, tag="ps", name="vps")
                    for kk in range(EK):
                        nc.tensor.matmul(
                            vps,
                            xT1[kk][:, st * 128 : (st + 1) * 128],
                            wv_sb[:, kk, :],
                            start=(kk == 0), stop=(kk == EK - 1),
                        )
                    for hh in range(2):
                        vaug = kvloc.tile([128, 65], BF16, tag="vaug")
                        nc.vector.tensor_copy(
                            out=vaug[:, 0:64], in_=vps[:, hh * 64 : hh * 64 + 64]
                        )
                        nc.vector.memset(vaug[:, 64:65], 1.0)
                        nc.sync.dma_start(out=v_in[l][p, hh, st], in_=vaug)
            allgather(v_in[l], v_out[l])

            # ---- attention ----------------------------------------------
            cat_tiles = []
            for p in range(NP):
                # gathered k^T for this pair: [128, S] over core blocks
                k_sb = kp.tile([128, c.NC, SL], BF16, tag="k_sb")
                nc.sync.dma_start(
                    out=k_sb, in_=k_out[l][:, p].transpose([1, 0, 2])
                )
                k_flat = k_sb.rearrange("p c s -> p (c s)")
                for hh in range(2):
                    h_idx = 2 * p + hh
                    hs = slice(hh * 64, hh * 64 + 64)
                    v_sb = vp.tile([128, c.NC, ST, 65], BF16, tag="v_sb")
                    for t2 in range(ST):
                        nc.sync.dma_start(
                            out=v_sb[:, :, t2, :],
                            in_=v_out[l][:, p, hh, t2].transpose([1, 0, 2]),
                        )
                    v_flat = v_sb.rearrange("p c t f -> p (c t) f")
                    avps = psp.tile([65, SL], F32, tag="ps", name="avps")
                    e_sb = ep.tile([128, TT, SL], BF16, tag="e")
                    for tp2 in range(TT // 2):
                        scps = psp.tile([128, 2 * SL], F32, tag="ps", name="scps")
                        for half in range(2):
                            tt = 2 * tp2 + half
                            nc.tensor.matmul(
                                scps[:, half * SL : (half + 1) * SL],
                                k_flat[hs, tt * 128 : (tt + 1) * 128],
                                q_sb[p][hs, :],
                            )
                        nc.scalar.activation(
                            out=e_sb[:, 2 * tp2 : 2 * tp2 + 2, :], in_=scps,
                            func=AF.Exp, scale=1.0 / np.sqrt(c.HS),
                        )
                        for half in range(2):
                            tt = 2 * tp2 + half
                            nc.tensor.matmul(
                                avps, v_flat[:, tt, :], e_sb[:, tt, :],
                                start=(tt == 0), stop=(tt == TT - 1),
                            )
                    # rowsums -> reciprocal -> partition-broadcast (SBUF DMA)
                    rs = rsp.tile([128, SL], F32, tag="rs")
                    nc.vector.reciprocal(out=rs[64:65, :], in_=avps[64:65, :])
                    rsb = rsp.tile([128, SL], BF16, tag="rsb")
                    nc.vector.tensor_copy(out=rsb[64:65, :], in_=rs[64:65, :])
                    rb = rbp.tile([128, SL], BF16, tag="rb")
                    nc.sync.dma_start(out=rb[0:1, :], in_=rsb[64:65, :])
                    nc.gpsimd.partition_broadcast(rb[:, :], rb[0:1, :])
                    # normalized av -> cat tile
                    cat = catp.tile([64, SL], BF16, tag="cat")
                    nc.vector.tensor_mul(out=cat, in0=avps[0:64, :], in1=rb[0:64, :])
                    cat_tiles.append(cat)
                    # normalized att map: in-place bcast multiply + one DMA
                    # (av matmuls finished reading e_sb before rb exists)
                    rb_rep = bass.AP(
                        tensor=rb.tensor,
                        offset=rb.offset,
                        ap=[list(rb.ap[0]), [0, TT]]
                        + [list(d) for d in rb.ap[1:]],
                    )
                    nc.vector.tensor_mul(out=e_sb, in0=e_sb, in1=rb_rep)
                    nc.sync.dma_start(
                        out=att_d[l, h_idx].rearrange("(t p) s -> p t s", p=128),
                        in_=e_sb,
                    )

            # ---- resize projection + residual ---------------------------
            for st in range(ST):
                nc.vector.tensor_add(out=xn1[st], in0=xn1[st], in1=br_bc)
            NN = c.E // 512 if c.E >= 512 else 1
            NW = min(c.E, 512)
            xatt_ps = [
                [psp.tile([128, NW], F32, tag="ps", name="xatt_ps")
                 for _ in range(NN)]
                for _ in range(ST)
            ]
            for h in range(c.H):
                for nn2 in range(NN):
                    wr_sb = wmisc.tile([64, NW], BF16, tag="wr_sb")
                    nc.sync.dma_start(
                        out=wr_sb,
                        in_=wr_d[l, h * 64 : h * 64 + 64, nn2 * NW : (nn2 + 1) * NW],
                    )
                    for st in range(ST):
                        nc.tensor.matmul(
                            xatt_ps[st][nn2],
                            cat_tiles[h][:, st * 128 : (st + 1) * 128],
                            wr_sb,
                            start=(h == 0), stop=(h == c.H - 1),
                        )
            x2_tiles = []
            for st in range(ST):
                x2 = xA.tile([128, c.E], F32, tag="x")
                for nn2 in range(NN):
                    nc.vector.tensor_add(
                        out=x2[:, nn2 * NW : (nn2 + 1) * NW],
                        in0=xatt_ps[st][nn2],
                        in1=xn1[st][:, nn2 * NW : (nn2 + 1) * NW],
                    )
                x2_tiles.append(x2)

            # ---- LN2 + FFN ----------------------------------------------
            xn2 = layernorm(x2_tiles, g_bc, b_bc)
            xT2 = transpose_to(xn2)
            for st in range(ST):
                nc.vector.tensor_add(out=xn2[st], in0=xn2[st], in1=b2_bc)

            y_ps = [
                [psp.tile([128, NW], F32, tag="ps", name="y_ps")
                 for _ in range(NN)]
                for _ in range(ST)
            ]
            for mk in range(FK):
                w1_sb = wmisc.tile([128, EK, 128], BF16, tag="w1_sb")
                nc.sync.dma_start(
                    out=w1_sb,
                    in_=w1_d[l, :, mk * 128 : (mk + 1) * 128].rearrange(
                        "(k p) m -> p k m", p=128
                    ),
                )
                hps = psp.tile([128, SL], F32, tag="ps", name="hps")
                for kk in range(EK):
                    nc.tensor.matmul(
                        hps, w1_sb[:, kk, :], xT2[kk],
                        start=(kk == 0), stop=(kk == EK - 1),
                    )
                h_sb = hp.tile([128, SL], BF16, tag="h_sb")
                nc.scalar.activation(
                    out=h_sb, in_=hps, func=AF.Relu,
                    bias=b1_sb[:, mk : mk + 1], scale=1.0,
                )
                w2_sb = wmisc.tile([128, c.E], BF16, tag="w2_sb")
                nc.sync.dma_start(
                    out=w2_sb, in_=w2_d[l, mk * 128 : (mk + 1) * 128, :]
                )
                for st in range(ST):
                    for nn2 in range(NN):
                        nc.tensor.matmul(
                            y_ps[st][nn2],
                            h_sb[:, st * 128 : (st + 1) * 128],
                            w2_sb[:, nn2 * NW : (nn2 + 1) * NW],
                            start=(mk == 0), stop=(mk == FK - 1),
                        )
            new_x = []
            for st in range(ST):
                x3 = xA.tile([128, c.E], F32, tag="x")
                for nn2 in range(NN):
                    nc.vector.tensor_add(
                        out=x3[:, nn2 * NW : (nn2 + 1) * NW],
                        in0=y_ps[st][nn2],
                        in1=xn2[st][:, nn2 * NW : (nn2 + 1) * NW],
                    )
                new_x.append(x3)
            x_tiles = new_x

        for st in range(ST):
            nc.sync.dma_start(
                out=xout_d[st * 128 : (st + 1) * 128, :], in_=x_tiles[st]
            )

    return nc


# --------------------------------------------------------------------------
# host side
# --------------------------------------------------------------------------

def make_in_maps(cfg: Cfg, seq, tok_emb, pos_emb, Wq, Wk, Wv, Wr, br, W1, b1,
                 W2, b2, ln_g, ln_b):
    c = cfg
    bf = ml_dtypes.bfloat16
    seq = np.asarray(seq)
    x0 = np.asarray(tok_emb)[seq.astype(np.int64)] + np.asarray(pos_emb)
    x0 = np.ascontiguousarray(x0, dtype=np.float32)

    # head-pair packed projection weights [L, NP, E, 2*HS]:
    # pair p columns = [head 2p | head 2p+1]
    def pairs(W):
        W = np.asarray(W)  # [L, H, E, HS]
        W = W.reshape(c.L, c.NP, 2, c.E, c.HS)
        W = np.concatenate([W[:, :, 0], W[:, :, 1]], axis=-1)  # [L, NP, E, 128]
        return np.ascontiguousarray(W, dtype=bf)

    common = {
        "wq": pairs(Wq),
        "wk": pairs(Wk),
        "wv": pairs(Wv),
        "wr": np.ascontiguousarray(np.asarray(Wr), dtype=bf),
        "w1": np.ascontiguousarray(np.asarray(W1), dtype=bf),
        "w2": np.ascontiguousarray(np.asarray(W2), dtype=bf),
        "lng": np.ascontiguousarray(np.asarray(ln_g), dtype=np.float32),
        "lnb": np.ascontiguousarray(np.asarray(ln_b), dtype=np.float32),
        "br": np.ascontiguousarray(np.asarray(br), dtype=np.float32),
        "b1": np.ascontiguousarray(np.asarray(b1), dtype=np.float32),
        "b2": np.ascontiguousarray(np.asarray(b2), dtype=np.float32),
    }
    in_maps = []
    for core in range(c.NC):
        m = dict(common)
        m["x0"] = x0[core * c.SL : (core + 1) * c.SL]
        in_maps.append(m)
    return in_maps


def assemble_outputs(cfg: Cfg, results):
    c = cfg
    x = np.concatenate(
        [np.asarray(r["x_out"], np.float32).reshape(c.SL, c.E) for r in results],
        axis=0,
    )
    att = np.empty((c.L, c.H, c.S, c.S), np.float32)
    for core, r in enumerate(results):
        blk = np.asarray(r["att_t"], np.float32).reshape(c.L, c.H, c.S, c.SL)
        att[:, :, core * c.SL : (core + 1) * c.SL, :] = blk.transpose(0, 1, 3, 2)
    return x, att


_CACHE = {}


def _get_nc(cfg: Cfg):
    key = cfg
    if key not in _CACHE:
        nc = bacc.Bacc(
            "TRN2",
            target_bir_lowering=False,
            debug=False,
            num_devices=cfg.NC,
        )
        build_encoder(nc, cfg)
        nc.compile()
        _CACHE[key] = nc
    return _CACHE[key]


LAST_RESULTS = {}


def kernel(seq, tok_emb, pos_emb, Wq, Wk, Wv, Wr, br, W1, b1, W2, b2, ln_g,
           ln_b):
    import os
    from concourse import bass_utils

    cfg = CFG_FULL
    nc = _get_nc(cfg)
    in_maps = make_in_maps(
        cfg, seq, tok_emb, pos_emb, Wq, Wk, Wv, Wr, br, W1, b1, W2, b2,
        ln_g, ln_b,
    )
    trace = bool(int(os.environ.get("KERNEL_TRACE", "0")))
    res = bass_utils.run_bass_kernel_spmd(
        nc, in_maps, core_ids=list(range(cfg.NC)), trace=trace
    )
    LAST_RESULTS["exec_time_ns"] = res.exec_time_ns
    LAST_RESULTS["profile_json"] = res.profile_json
    return assemble_outputs(cfg, res.results)
